# revision 1
# baseline (speedup 1.0000x reference)
"""Trainium2 Bass kernel for the Covid compartment forecast model.

Computation (per posterior sample s):
    growth[t,s] = r_t[t]**(1/T_serial[s]) * delta[s]
    A[t,s]      = A[t-1,s] * growth[t,s]            (scan, A[-1] = warmup[-1])
    A_full      = concat(warmup, A)                 # [J+T, S]
    M[t,s]      = sum_j A_full[J-1-j+t, s] * rho[s] * pi[j, s]

Sharding: posterior-sample dimension S across 8 cores (pure data parallel).
On-chip layout: samples on partitions (tiles of 128), time on the free dim.

Engine plan per 128-sample tile:
  ACT : g = Exp(log_r * invT + ln_delta)   (per-partition scale/bias)
  DVE : A = tensor_tensor_scan(mult)       (the day recursion)
  FIR (32 taps) split across engines:
    PE   : diag(q_j) matmuls accumulated in PSUM
    ACT  : scaled copies B_j = A_shift * q_j, identity-matmul-accumulated by PE
    DVE  : scalar_tensor_tensor fused MACs into an SBUF accumulator
    Pool : scalar_tensor_tensor fused MACs into an SBUF accumulator
  PE   : 128x128 output transposes ([s,t] -> [t,s]), DMA from PSUM to DRAM.
"""

import numpy as np

import concourse.bacc as bacc
import concourse.bass as bass
import concourse.mybir as mybir
import concourse.tile as tile
from concourse.bass_utils import run_bass_kernel_spmd

F32 = mybir.dt.float32
F32R = mybir.dt.float32r
I32 = mybir.dt.int32
AF = mybir.ActivationFunctionType
OP = mybir.AluOpType

T = 1024
J = 32
S_TOTAL = 50000
NCORES = 8
P = 128
S_CORE = S_TOTAL // NCORES           # 6250
NTILES = (S_CORE + P - 1) // P       # 49
S_PAD = NTILES * P                   # 6272

# Tap assignment: which lag j is handled by which engine.
PE_TAPS = tuple(range(0, 8))
ACT_TAPS = tuple(range(8, 16))
POOL_TAPS = tuple(range(16, 22))
DVE_TAPS = tuple(range(22, 32))


def build(s_pad=S_PAD, pe_taps=PE_TAPS, act_taps=ACT_TAPS, dve_taps=DVE_TAPS,
          pool_taps=POOL_TAPS):
    assert s_pad % P == 0
    ntiles = s_pad // P
    taps = sorted(list(pe_taps) + list(act_taps) + list(dve_taps) + list(pool_taps))
    assert taps == list(range(J)), f"tap assignment must cover 0..{J-1}: {taps}"

    nc = bacc.Bacc("TRN2", target_bir_lowering=False, debug=False,
                   num_devices=NCORES)
    r = nc.dram_tensor("r_t", [1, T], F32, kind="ExternalInput").ap()
    wu = nc.dram_tensor("warmup", [J, s_pad], F32, kind="ExternalInput").ap()
    pi = nc.dram_tensor("pi", [J, s_pad], F32, kind="ExternalInput").ap()
    dl = nc.dram_tensor("delta", [1, s_pad], F32, kind="ExternalInput").ap()
    ts = nc.dram_tensor("t_serial", [1, s_pad], F32, kind="ExternalInput").ap()
    rh = nc.dram_tensor("rho", [1, s_pad], F32, kind="ExternalInput").ap()
    m = nc.dram_tensor("m_out", [T, s_pad], F32, kind="ExternalOutput").ap()

    # [1, s_pad] DRAM param -> [P, ntiles] SBUF layout: (p, i) = param[i*P + p]
    def param_ap(a):
        return bass.AP(tensor=a.tensor, offset=a.offset,
                       ap=[[1, P], [P, ntiles]])

    with tile.TileContext(nc) as tc:
        with (
            tc.tile_pool(name="singles", bufs=1) as singles,
            tc.tile_pool(name="loads", bufs=4) as loads,
            tc.tile_pool(name="apool", bufs=2) as apool,
            tc.tile_pool(name="gpool", bufs=2) as gpool,
            tc.tile_pool(name="qpool", bufs=4) as qpool,
            tc.tile_pool(name="diags", bufs=4) as diags,
            tc.tile_pool(name="bpool", bufs=3) as bpool,
            tc.tile_pool(name="mdp", bufs=2) as mdp,
            tc.tile_pool(name="msb", bufs=2) as msb,
            tc.tile_pool(name="mtp", bufs=2) as mtp,
            tc.tile_pool(name="mpsum", bufs=2, space="PSUM") as mpsum,
            tc.tile_pool(name="trpsum", bufs=4, space="PSUM") as trpsum,
        ):
            # ---- one-time setup ----
            iota_t = singles.tile([P, P], I32)
            nc.gpsimd.iota(iota_t, pattern=[[1, P]], base=0,
                           channel_multiplier=-1)
            ident = singles.tile([P, P], F32)
            nc.vector.tensor_scalar(out=ident, in0=iota_t, scalar1=0,
                                    scalar2=None, op0=OP.is_equal)
            ident_r = singles.tile([P, P], F32R)
            nc.vector.tensor_scalar(out=ident_r, in0=iota_t, scalar1=0,
                                    scalar2=None, op0=OP.is_equal)

            # broadcast r_t to all partitions (stride-0 DRAM read), then Ln
            r_bc = singles.tile([P, T], F32)
            nc.sync.dma_start(
                out=r_bc,
                in_=bass.AP(tensor=r.tensor, offset=r.offset,
                            ap=[[0, P], [1, T]]))
            lr_bc = singles.tile([P, T], F32)
            nc.scalar.activation(out=lr_bc, in_=r_bc, func=AF.Ln)

            dl_sb = singles.tile([P, ntiles], F32)
            nc.sync.dma_start(out=dl_sb, in_=param_ap(dl))
            ts_sb = singles.tile([P, ntiles], F32)
            nc.sync.dma_start(out=ts_sb, in_=param_ap(ts))
            rh_sb = singles.tile([P, ntiles], F32)
            nc.sync.dma_start(out=rh_sb, in_=param_ap(rh))

            # ---- per sample-tile ----
            for i in range(ntiles):
                s0 = i * P

                wu_nat = loads.tile([J, P], F32, tag="wu")
                nc.sync.dma_start(out=wu_nat, in_=wu[:, s0:s0 + P])
                pi_nat = loads.tile([J, P], F32, tag="pi")
                nc.sync.dma_start(out=pi_nat, in_=pi[:, s0:s0 + P])

                A_full = apool.tile([P, J + T], F32R)

                wuT = trpsum.tile([P, P], F32, tag="tr")
                nc.tensor.transpose(wuT[:, 0:J], wu_nat, ident[0:J, 0:J])
                nc.scalar.copy(out=A_full[:, 0:J], in_=wuT[:, 0:J])

                piT = trpsum.tile([P, P], F32, tag="tr")
                nc.tensor.transpose(piT[:, 0:J], pi_nat, ident[0:J, 0:J])
                q = qpool.tile([P, J], F32, tag="q")
                nc.vector.tensor_scalar(out=q, in0=piT[:, 0:J],
                                        scalar1=rh_sb[:, i:i + 1],
                                        scalar2=None, op0=OP.mult)

                invT = qpool.tile([P, 1], F32, tag="invT")
                nc.vector.reciprocal(out=invT, in_=ts_sb[:, i:i + 1])
                lnd = qpool.tile([P, 1], F32, tag="lnd")
                nc.scalar.activation(out=lnd, in_=dl_sb[:, i:i + 1], func=AF.Ln)

                g = gpool.tile([P, T], F32)
                nc.scalar.activation(out=g, in_=lr_bc, func=AF.Exp,
                                     bias=lnd, scale=invT)

                # A[t] = A[t-1] * g[t], A[-1] = warmup[:, J-1]
                nc.vector.tensor_tensor_scan(
                    out=A_full[:, J:J + T], data0=g, data1=g,
                    initial=A_full[:, J - 1:J], op0=OP.mult, op1=OP.bypass)

                # ---- FIR: M[t] = sum_j q[j] * A_full[31-j+t] ----
                # Only TensorE matmuls touch PSUM has_written, so every
                # contribution goes through the PE accumulation group.
                Mp = mpsum.tile([P, T], F32, tag="Mp")
                chunk_started = [False, False]

                def pe_acc(lhsT, rhs_base, stop=False):
                    for c in range(2):
                        lo = c * 512
                        nc.tensor.matmul(
                            Mp[:, lo:lo + 512], lhsT,
                            rhs_base[:, lo:lo + 512],
                            start=not chunk_started[c], stop=stop)
                        chunk_started[c] = True

                act_rest = act_taps
                for j in pe_taps:
                    dg = diags.tile([P, P], F32R, tag="diag")
                    nc.vector.tensor_scalar(out=dg, in0=ident,
                                            scalar1=q[:, j:j + 1],
                                            scalar2=None, op0=OP.mult)
                    pe_acc(dg, A_full[:, J - 1 - j:J - 1 - j + T])

                for j in act_rest:
                    B = bpool.tile([P, T], F32R, tag="b")
                    nc.scalar.activation(out=B,
                                         in_=A_full[:, J - 1 - j:J - 1 - j + T],
                                         func=AF.Copy, scale=q[:, j:j + 1])
                    pe_acc(ident_r, B)

                for j in pool_taps:
                    B = bpool.tile([P, T], F32R, tag="b")
                    nc.gpsimd.tensor_scalar(out=B,
                                            in0=A_full[:, J - 1 - j:J - 1 - j + T],
                                            scalar1=q[:, j:j + 1],
                                            scalar2=None, op0=OP.mult)
                    pe_acc(ident_r, B)

                Md = mdp.tile([P, T], F32R)
                for k, j in enumerate(dve_taps):
                    sh = A_full[:, J - 1 - j:J - 1 - j + T]
                    if k == 0:
                        nc.vector.tensor_scalar(out=Md, in0=sh,
                                                scalar1=q[:, j:j + 1],
                                                scalar2=None, op0=OP.mult)
                    else:
                        nc.vector.scalar_tensor_tensor(
                            out=Md, in0=sh, scalar=q[:, j:j + 1], in1=Md,
                            op0=OP.mult, op1=OP.add)
                pe_acc(ident_r, Md, stop=True)

                M_sb = msb.tile([P, T], F32)
                nc.scalar.copy(out=M_sb, in_=Mp)

                # transpose out: [s, t] tiles -> [t, s] DRAM
                # Mt_sb[:, b*P:(b+1)*P] holds M rows [b*P, (b+1)*P) of this
                # sample block; one strided DMA scatters all 8 blocks.
                Mt_sb = mtp.tile([P, T], F32)
                for b in range(T // P):
                    mt = trpsum.tile([P, P], F32, tag="tr")
                    nc.tensor.transpose(mt, M_sb[:, b * P:(b + 1) * P], ident)
                    nc.scalar.copy(out=Mt_sb[:, b * P:(b + 1) * P], in_=mt)
                out_ap = bass.AP(tensor=m.tensor, offset=m.offset + s0,
                                 ap=[[s_pad, P], [P * s_pad, T // P], [1, P]])
                nc.sync.dma_start(out=out_ap, in_=Mt_sb)

    nc.compile()
    return nc


_NC_CACHE = {}


def _get_nc():
    key = (S_PAD, PE_TAPS, ACT_TAPS, DVE_TAPS, POOL_TAPS)
    if key not in _NC_CACHE:
        _NC_CACHE[key] = build()
    return _NC_CACHE[key]


def _shard_inputs(r_t, warmup_A, delta, T_serial, rho_M, pi_M):
    """Slice the full inputs per core and pad S_CORE -> S_PAD."""
    r2 = np.ascontiguousarray(r_t, dtype=np.float32).reshape(1, T)
    in_maps = []
    for c in range(NCORES):
        lo, hi = c * S_CORE, (c + 1) * S_CORE
        pad = S_PAD - S_CORE

        def pad2(a, fill):
            a = np.asarray(a, dtype=np.float32)[:, lo:hi]
            return np.pad(a, ((0, 0), (0, pad)), constant_values=fill)

        def pad1(a, fill):
            a = np.asarray(a, dtype=np.float32)[lo:hi].reshape(1, -1)
            return np.pad(a, ((0, 0), (0, pad)), constant_values=fill)

        in_maps.append({
            "r_t": r2,
            "warmup": pad2(warmup_A, 1.0),
            "pi": pad2(pi_M, 1.0 / J),
            "delta": pad1(delta, 1.0),
            "t_serial": pad1(T_serial, 5.0),
            "rho": pad1(rho_M, 0.0),
        })
    return in_maps


def run(inputs, trace=False, **kwargs):
    """Run on 8 cores; returns (M [T, S_TOTAL] float32, BassKernelResults)."""
    nc = _get_nc()
    in_maps = _shard_inputs(**inputs)
    res = run_bass_kernel_spmd(nc, in_maps, core_ids=list(range(NCORES)),
                               trace=trace, **kwargs)
    M = np.concatenate(
        [res.results[c]["m_out"][:, :S_CORE] for c in range(NCORES)], axis=1)
    return np.ascontiguousarray(M, dtype=np.float32), res


def kernel(**inputs):
    M, _ = run(inputs)
    return M



# revision 2
# speedup vs baseline: 8.6405x; 8.6405x over previous
"""Trainium2 Bass kernel for the Covid compartment forecast model.

Math (per posterior sample s, forecast day t in [0, T)):
    growth[t,s] = r_t[t]**(1/T_serial[s]) * delta[s]
    A[t,s]      = A[t-1,s] * growth[t,s],  A[-1] = warmup[-1]
    M[t,s]      = sum_j A_full[J-1-j+t, s] * rho[s] * pi[j, s]

The sequential scan is replaced by its closed form
    A[t,s] = exp(b[s] + (t+1)*ln(delta[s]) + invT[s] * L[t]),
    L[t] = cumsum(ln r_t)[t],  b[s] = ln(warmup[-1, s])
so each 128-sample tile is fully parallel: one DVE scalar_tensor_tensor
builds the exponent, one ACT Exp (per-partition scale/bias) emits A in
bf16. The 32-tap FIR runs in bf16 split across engines by measured cost:
  PE   : diag(q_j) matmuls accumulated in PSUM        (~432ns/tap)
  ACT  : scaled copies B_j = A_shift * q_j            (~1131ns/tap)
  DVE  : tensor_scalar product + tensor_tensor adds   (~928ns/tap)
ACT products are pair-summed on DVE, then everything funnels through
identity matmuls into the same PSUM accumulation. M leaves as bf16 in
[sample, day] layout; the host does the final transpose/concat.

Sharding: posterior-sample dimension S across 8 cores (data parallel).
"""

import numpy as np

import concourse.bacc as bacc
import concourse.bass as bass
import concourse.mybir as mybir
import concourse.tile as tile
from concourse.bass_utils import run_bass_kernel_spmd

F32 = mybir.dt.float32
BF16 = mybir.dt.bfloat16
I32 = mybir.dt.int32
AF = mybir.ActivationFunctionType
OP = mybir.AluOpType

T = 1024
J = 32
S_TOTAL = 50000
NCORES = 8
P = 128
S_CORE = S_TOTAL // NCORES           # 6250
NTILES = (S_CORE + P - 1) // P       # 49
S_PAD = NTILES * P                   # 6272

# Tap -> engine. Offsets are 31-j; DVE taps take odd j (even offsets,
# needed for the 4x tensor_scalar perf mode on bf16).
PE_TAPS = tuple(range(0, 20))
ACT_TAPS = (20, 22, 24, 26, 28, 30)
DVE_TAPS = (21, 23, 25, 27, 29, 31)


def build():
    taps = sorted(PE_TAPS + ACT_TAPS + DVE_TAPS)
    assert taps == list(range(J))

    nc = bacc.Bacc("TRN2", target_bir_lowering=False, debug=False,
                   num_devices=NCORES)
    it1 = nc.dram_tensor("iota1", [1, T], F32, kind="ExternalInput").ap()
    lc = nc.dram_tensor("lcum", [1, T], F32, kind="ExternalInput").ap()
    q = nc.dram_tensor("q", [S_PAD, J], F32, kind="ExternalInput").ap()
    wu = nc.dram_tensor("wu", [S_PAD, J], F32, kind="ExternalInput").ap()
    sc = nc.dram_tensor("scal", [S_PAD, 4], F32, kind="ExternalInput").ap()
    m = nc.dram_tensor("m_out", [S_PAD, T], BF16, kind="ExternalOutput").ap()

    def bcast(a):
        return bass.AP(tensor=a.tensor, offset=a.offset, ap=[[0, P], [1, T]])

    with tile.TileContext(nc) as tc:
        with (
            tc.tile_pool(name="singles", bufs=1) as singles,
            tc.tile_pool(name="loads", bufs=3) as loads,
            tc.tile_pool(name="argp", bufs=2) as argp,
            tc.tile_pool(name="apool", bufs=2) as apool,
            tc.tile_pool(name="diags", bufs=4) as diags,
            tc.tile_pool(name="bpool", bufs=4) as bpool,
            tc.tile_pool(name="cpool", bufs=3) as cpool,
            tc.tile_pool(name="mdp", bufs=2) as mdp,
            tc.tile_pool(name="msb", bufs=2) as msb,
            tc.tile_pool(name="mpsum", bufs=2, space="PSUM") as mpsum,
        ):
            # ---- one-time setup ----
            iota_t = singles.tile([P, P], I32)
            nc.gpsimd.iota(iota_t, pattern=[[1, P]], base=0,
                           channel_multiplier=-1)
            ident = singles.tile([P, P], BF16)
            nc.vector.tensor_scalar(out=ident, in0=iota_t, scalar1=0,
                                    scalar2=None, op0=OP.is_equal)
            it_bc = singles.tile([P, T], F32)
            nc.sync.dma_start(out=it_bc, in_=bcast(it1))
            L_bc = singles.tile([P, T], F32)
            nc.sync.dma_start(out=L_bc, in_=bcast(lc))

            # ---- per sample-tile ----
            for i in range(NTILES):
                s0 = i * P

                qt = loads.tile([P, J], F32, tag="q")
                nc.sync.dma_start(out=qt, in_=q[s0:s0 + P, :])
                wut = loads.tile([P, J], F32, tag="wu")
                nc.sync.dma_start(out=wut, in_=wu[s0:s0 + P, :])
                sct = loads.tile([P, 4], F32, tag="sc")
                nc.sync.dma_start(out=sct, in_=sc[s0:s0 + P, :])

                # exponent: arg[s,t] = (t+1)*w[s] + L[t]   (w = lnd*T_serial)
                argt = argp.tile([P, T], F32)
                nc.vector.scalar_tensor_tensor(
                    out=argt, in0=it_bc, scalar=sct[:, 0:1], in1=L_bc,
                    op0=OP.mult, op1=OP.add)

                # A_full[:, 0:J] = warmup (bf16), A_full[:, J:] = exp(...)
                A = apool.tile([P, J + T], BF16)
                nc.vector.tensor_copy(A[:, 0:J], wut)
                nc.scalar.activation(out=A[:, J:J + T], in_=argt, func=AF.Exp,
                                     bias=sct[:, 2:3], scale=sct[:, 1:2])

                # ---- FIR: M[t] = sum_j q[j] * A_full[31-j+t] ----
                Mp = mpsum.tile([P, T], F32)
                started = [False, False]

                def pe_acc(lhsT, rhs_base, stop=False):
                    for c in range(2):
                        lo = c * 512
                        nc.tensor.matmul(
                            Mp[:, lo:lo + 512], lhsT,
                            rhs_base[:, lo:lo + 512],
                            start=not started[c], stop=stop)
                        started[c] = True

                for j in PE_TAPS:
                    dg = diags.tile([P, P], BF16, tag="dg")
                    nc.vector.tensor_scalar(out=dg, in0=ident,
                                            scalar1=qt[:, j:j + 1],
                                            scalar2=None, op0=OP.mult)
                    pe_acc(dg, A[:, J - 1 - j:J - 1 - j + T])

                # ACT taps -> pairwise sums on DVE -> PE merge
                Bs = []
                for j in ACT_TAPS:
                    B = bpool.tile([P, T], BF16, tag="b")
                    nc.scalar.activation(out=B,
                                         in_=A[:, J - 1 - j:J - 1 - j + T],
                                         func=AF.Copy, scale=qt[:, j:j + 1])
                    Bs.append(B)
                for k in range(0, len(Bs), 2):
                    C = cpool.tile([P, T], BF16, tag="c")
                    nc.vector.tensor_tensor(out=C, in0=Bs[k], in1=Bs[k + 1],
                                            op=OP.add)
                    pe_acc(ident, C)

                # DVE taps: TS product + TT adds into Md
                Md = mdp.tile([P, T], BF16, tag="md")
                for k, j in enumerate(DVE_TAPS):
                    sh = A[:, J - 1 - j:J - 1 - j + T]
                    if k == 0:
                        nc.vector.tensor_scalar(out=Md, in0=sh,
                                                scalar1=qt[:, j:j + 1],
                                                scalar2=None, op0=OP.mult)
                    else:
                        Bt = mdp.tile([P, T], BF16, tag="bt")
                        nc.vector.tensor_scalar(out=Bt, in0=sh,
                                                scalar1=qt[:, j:j + 1],
                                                scalar2=None, op0=OP.mult)
                        nc.vector.tensor_tensor(out=Md, in0=Md, in1=Bt,
                                                op=OP.add)
                pe_acc(ident, Md, stop=True)

                # PSUM -> SBUF bf16 (split DVE/ACT), then straight DMA out
                M_sb = msb.tile([P, T], BF16)
                nc.vector.tensor_copy(M_sb[:, 0:512], Mp[:, 0:512])
                nc.scalar.activation(out=M_sb[:, 512:1024],
                                     in_=Mp[:, 512:1024], func=AF.Copy)
                nc.sync.dma_start(out=m[s0:s0 + P, :], in_=M_sb)

    nc.compile()
    return nc


_NC_CACHE = {}


def _get_nc():
    key = (S_PAD, PE_TAPS, ACT_TAPS, DVE_TAPS)
    if key not in _NC_CACHE:
        _NC_CACHE[key] = build()
    return _NC_CACHE[key]


def _prep_inputs(r_t, warmup_A, delta, T_serial, rho_M, pi_M):
    """Host-side parameter prep + per-core sharding along S."""
    r_t = np.asarray(r_t, dtype=np.float32)
    warmup_A = np.asarray(warmup_A, dtype=np.float32)
    delta = np.asarray(delta, dtype=np.float32)
    T_serial = np.asarray(T_serial, dtype=np.float32)
    rho_M = np.asarray(rho_M, dtype=np.float32)
    pi_M = np.asarray(pi_M, dtype=np.float32)

    iota1 = np.arange(1, T + 1, dtype=np.float32).reshape(1, T)
    lcum = np.cumsum(np.log(r_t), dtype=np.float32).reshape(1, T)
    lnd = np.log(delta)
    q_full = (rho_M[None, :] * pi_M).T.astype(np.float32)       # [S, J]
    wu_full = warmup_A.T.astype(np.float32)                      # [S, J]
    w_full = lnd * T_serial
    invT_full = (1.0 / T_serial).astype(np.float32)
    b_full = np.log(warmup_A[-1]).astype(np.float32)

    pad = S_PAD - S_CORE
    in_maps = []
    for c in range(NCORES):
        lo, hi = c * S_CORE, (c + 1) * S_CORE

        def pad2(a, fill):
            return np.pad(a[lo:hi], ((0, pad), (0, 0)), constant_values=fill)

        scal = np.stack([w_full[lo:hi], invT_full[lo:hi], b_full[lo:hi],
                         np.zeros(S_CORE, np.float32)], axis=1)
        # padded lanes: w=-1, invT=1, b=0 -> A decays, q=0 -> M=0
        scal = np.pad(scal, ((0, pad), (0, 0)), constant_values=0.0)
        scal[S_CORE:, 0] = -1.0
        scal[S_CORE:, 1] = 1.0

        in_maps.append({
            "iota1": iota1,
            "lcum": lcum,
            "q": pad2(q_full, 0.0),
            "wu": pad2(wu_full, 1.0),
            "scal": np.ascontiguousarray(scal),
        })
    return in_maps


def _bf16_to_f32(a):
    a = np.asarray(a)
    if a.dtype == np.float32:
        return a
    u = a.view(np.uint16).astype(np.uint32) << 16
    return u.view(np.float32)


def run(inputs, trace=False, **kwargs):
    """Run on 8 cores; returns (M [T, S_TOTAL] float32, BassKernelResults)."""
    nc = _get_nc()
    in_maps = _prep_inputs(**inputs)
    res = run_bass_kernel_spmd(nc, in_maps, core_ids=list(range(NCORES)),
                               trace=trace, **kwargs)
    cols = []
    for c in range(NCORES):
        mc = _bf16_to_f32(res.results[c]["m_out"])[:S_CORE]   # [S_CORE, T]
        cols.append(mc.T)
    M = np.concatenate(cols, axis=1)
    return np.ascontiguousarray(M, dtype=np.float32), res


def kernel(**inputs):
    M, _ = run(inputs)
    return M


# revision 5
# speedup vs baseline: 10.4478x; 1.2092x over previous
"""Trainium2 Bass kernel for the Covid compartment forecast model.

Math (per posterior sample s, forecast day t in [0, T)):
    growth[t,s] = r_t[t]**(1/T_serial[s]) * delta[s]
    A[t,s]      = A[t-1,s] * growth[t,s],  A[-1] = warmup[-1]
    M[t,s]      = sum_j A_full[J-1-j+t, s] * rho[s] * pi[j, s]

The sequential scan is replaced by its closed form
    A[t,s] = exp(b[s] + (t+1)*ln(delta[s]) + invT[s] * L[t]),
    L[t] = cumsum(ln r_t)[t],  b[s] = ln(warmup[-1, s])
so each 128-sample tile is fully parallel: one DVE scalar_tensor_tensor
builds the exponent, one ACT Exp (per-partition scale/bias) emits A in
bf16. The 32-tap FIR runs in bf16 split across engines by measured cost:
  PE   : diag(q_j) matmuls accumulated in PSUM        (~432ns/tap)
  ACT  : scaled copies B_j = A_shift * q_j            (~1131ns/tap)
  DVE  : tensor_scalar product + tensor_tensor adds   (~928ns/tap)
ACT products are pair-summed on DVE, then everything funnels through
identity matmuls into the same PSUM accumulation. M leaves as bf16 in
[sample, day] layout; the host does the final transpose/concat.

Sharding: posterior-sample dimension S across 8 cores (data parallel).
"""

import numpy as np

import concourse.bacc as bacc
import concourse.bass as bass
import concourse.mybir as mybir
import concourse.tile as tile
from concourse.bass_utils import run_bass_kernel_spmd

F32 = mybir.dt.float32
BF16 = mybir.dt.bfloat16
I32 = mybir.dt.int32
AF = mybir.ActivationFunctionType
OP = mybir.AluOpType

T = 1024
J = 32
S_TOTAL = 50000
NCORES = 8
P = 128
S_CORE = S_TOTAL // NCORES           # 6250
NTILES = (S_CORE + P - 1) // P       # 49
S_PAD = NTILES * P                   # 6272

# Tap -> engine. Offsets are 31-j; DVE taps take odd j (even offsets,
# needed for the 4x tensor_scalar perf mode on bf16). PE diag weight
# matrices are prebuilt on the host and DMA'd (DMA engines are idle).
PE_TAPS = tuple(range(0, 19))
ACT_TAPS = (20, 22, 24, 26, 28, 30)
DVE_TAPS = (19, 21, 23, 25, 27, 29, 31)


def build():
    taps = sorted(PE_TAPS + ACT_TAPS + DVE_TAPS)
    assert taps == list(range(J))

    nc = bacc.Bacc("TRN2", target_bir_lowering=False, debug=False,
                   num_devices=NCORES)
    it1 = nc.dram_tensor("iota1", [1, T], F32, kind="ExternalInput").ap()
    lc = nc.dram_tensor("lcum", [1, T], F32, kind="ExternalInput").ap()
    q = nc.dram_tensor("q", [S_PAD, J], F32, kind="ExternalInput").ap()
    wu = nc.dram_tensor("wu", [S_PAD, J], BF16, kind="ExternalInput").ap()
    sc = nc.dram_tensor("scal", [S_PAD, 4], F32, kind="ExternalInput").ap()
    qd = nc.dram_tensor("qdiag", [S_PAD, len(PE_TAPS) * P], BF16,
                        kind="ExternalInput").ap()
    m = nc.dram_tensor("m_out", [S_PAD, T], BF16, kind="ExternalOutput").ap()

    def bcast(a):
        return bass.AP(tensor=a.tensor, offset=a.offset, ap=[[0, P], [1, T]])

    with tile.TileContext(nc) as tc:
        with (
            tc.tile_pool(name="singles", bufs=1) as singles,
            tc.tile_pool(name="loads", bufs=3) as loads,
            tc.tile_pool(name="argp", bufs=2) as argp,
            tc.tile_pool(name="apool", bufs=2) as apool,
            tc.tile_pool(name="bpool", bufs=4) as bpool,
            tc.tile_pool(name="cpool", bufs=3) as cpool,
            tc.tile_pool(name="mdp", bufs=2) as mdp,
            tc.tile_pool(name="msb", bufs=2) as msb,
            tc.tile_pool(name="mpsum", bufs=2, space="PSUM") as mpsum,
        ):
            # ---- one-time setup ----
            iota_t = singles.tile([P, P], I32)
            nc.gpsimd.iota(iota_t, pattern=[[1, P]], base=0,
                           channel_multiplier=-1)
            ident = singles.tile([P, P], BF16)
            nc.vector.tensor_scalar(out=ident, in0=iota_t, scalar1=0,
                                    scalar2=None, op0=OP.is_equal)
            it_bc = singles.tile([P, T], F32)
            nc.sync.dma_start(out=it_bc, in_=bcast(it1))
            L_bc = singles.tile([P, T], F32)
            nc.sync.dma_start(out=L_bc, in_=bcast(lc))

            # ---- per sample-tile ----
            for i in range(NTILES):
                s0 = i * P

                qt = loads.tile([P, J], F32, tag="q")
                nc.sync.dma_start(out=qt, in_=q[s0:s0 + P, :])
                sct = loads.tile([P, 4], F32, tag="sc")
                nc.sync.dma_start(out=sct, in_=sc[s0:s0 + P, :])
                dgs = loads.tile([P, len(PE_TAPS) * P], BF16, tag="dgs")
                nc.sync.dma_start(out=dgs, in_=qd[s0:s0 + P, :])

                # exponent: arg[s,t] = (t+1)*w[s] + L[t]   (w = lnd*T_serial)
                argt = argp.tile([P, T], F32)
                nc.vector.scalar_tensor_tensor(
                    out=argt, in0=it_bc, scalar=sct[:, 0:1], in1=L_bc,
                    op0=OP.mult, op1=OP.add)

                # A_full[:, 0:J] = warmup (DMA, bf16), A_full[:, J:] = exp(...)
                A = apool.tile([P, J + T], BF16)
                nc.sync.dma_start(out=A[:, 0:J], in_=wu[s0:s0 + P, :])
                nc.scalar.activation(out=A[:, J:J + T], in_=argt, func=AF.Exp,
                                     bias=sct[:, 2:3], scale=sct[:, 1:2])

                # ---- FIR: M[t] = sum_j q[j] * A_full[31-j+t] ----
                Mp = mpsum.tile([P, T], F32)
                started = [False, False]

                def pe_acc(lhsT, rhs_base, stop=False):
                    for c in range(2):
                        lo = c * 512
                        nc.tensor.matmul(
                            Mp[:, lo:lo + 512], lhsT,
                            rhs_base[:, lo:lo + 512],
                            start=not started[c], stop=stop)
                        started[c] = True

                for k, j in enumerate(PE_TAPS):
                    pe_acc(dgs[:, k * P:(k + 1) * P],
                           A[:, J - 1 - j:J - 1 - j + T])

                # ACT taps -> pairwise sums on DVE -> PE merge
                Bs = []
                for j in ACT_TAPS:
                    B = bpool.tile([P, T], BF16, tag="b")
                    nc.scalar.activation(out=B,
                                         in_=A[:, J - 1 - j:J - 1 - j + T],
                                         func=AF.Copy, scale=qt[:, j:j + 1])
                    Bs.append(B)
                for k in range(0, len(Bs), 2):
                    C = cpool.tile([P, T], BF16, tag="c")
                    nc.vector.tensor_tensor(out=C, in0=Bs[k], in1=Bs[k + 1],
                                            op=OP.add)
                    pe_acc(ident, C)

                # DVE taps: TS products + balanced tree of TT adds
                prods = []
                for k, j in enumerate(DVE_TAPS):
                    Bt = mdp.tile([P, T], BF16, tag=f"bt{k}")
                    nc.vector.tensor_scalar(out=Bt,
                                            in0=A[:, J - 1 - j:J - 1 - j + T],
                                            scalar1=qt[:, j:j + 1],
                                            scalar2=None, op0=OP.mult)
                    prods.append(Bt)
                lvl = 0
                while len(prods) > 1:
                    nxt = []
                    for k in range(0, len(prods) - 1, 2):
                        Sm = mdp.tile([P, T], BF16, tag=f"s{lvl}{k}")
                        nc.vector.tensor_tensor(out=Sm, in0=prods[k],
                                                in1=prods[k + 1], op=OP.add)
                        nxt.append(Sm)
                    if len(prods) % 2:
                        nxt.append(prods[-1])
                    prods = nxt
                    lvl += 1
                pe_acc(ident, prods[0], stop=True)

                # PSUM -> SBUF bf16 on ACT (DVE is the bottleneck)
                M_sb = msb.tile([P, T], BF16)
                nc.scalar.activation(out=M_sb[:, 0:512],
                                     in_=Mp[:, 0:512], func=AF.Copy)
                nc.scalar.activation(out=M_sb[:, 512:1024],
                                     in_=Mp[:, 512:1024], func=AF.Copy)
                nc.sync.dma_start(out=m[s0:s0 + P, :], in_=M_sb)

    nc.compile()
    return nc


_NC_CACHE = {}


def _get_nc():
    key = (S_PAD, PE_TAPS, ACT_TAPS, DVE_TAPS)
    if key not in _NC_CACHE:
        _NC_CACHE[key] = build()
    return _NC_CACHE[key]


def _prep_inputs(r_t, warmup_A, delta, T_serial, rho_M, pi_M):
    """Host-side parameter prep + per-core sharding along S."""
    r_t = np.asarray(r_t, dtype=np.float32)
    warmup_A = np.asarray(warmup_A, dtype=np.float32)
    delta = np.asarray(delta, dtype=np.float32)
    T_serial = np.asarray(T_serial, dtype=np.float32)
    rho_M = np.asarray(rho_M, dtype=np.float32)
    pi_M = np.asarray(pi_M, dtype=np.float32)

    iota1 = np.arange(1, T + 1, dtype=np.float32).reshape(1, T)
    lcum = np.cumsum(np.log(r_t), dtype=np.float32).reshape(1, T)
    lnd = np.log(delta)
    q_full = (rho_M[None, :] * pi_M).T.astype(np.float32)       # [S, J]
    wu_full = warmup_A.T.astype(np.float32)                      # [S, J]
    w_full = lnd * T_serial
    invT_full = (1.0 / T_serial).astype(np.float32)
    b_full = np.log(warmup_A[-1]).astype(np.float32)

    q16 = np.right_shift(q_full.view(np.uint32) + 0x8000, 16).astype(np.uint16)

    pad = S_PAD - S_CORE
    in_maps = []
    for c in range(NCORES):
        lo, hi = c * S_CORE, (c + 1) * S_CORE

        def pad2(a, fill):
            return np.pad(a[lo:hi], ((0, pad), (0, 0)), constant_values=fill)

        scal = np.stack([w_full[lo:hi], invT_full[lo:hi], b_full[lo:hi],
                         np.zeros(S_CORE, np.float32)], axis=1)
        # padded lanes: w=-1, invT=1, b=0 -> A decays, q=0 -> M=0
        scal = np.pad(scal, ((0, pad), (0, 0)), constant_values=0.0)
        scal[S_CORE:, 0] = -1.0
        scal[S_CORE:, 1] = 1.0

        import ml_dtypes
        qd = np.zeros((S_PAD, len(PE_TAPS) * P), dtype=np.uint16)
        idx = np.arange(S_CORE)
        for k, j in enumerate(PE_TAPS):
            qd[idx, k * P + (idx % P)] = q16[lo:hi, j]
        in_maps.append({
            "iota1": iota1,
            "lcum": lcum,
            "q": pad2(q_full, 0.0),
            "wu": np.right_shift(
                pad2(wu_full, 1.0).view(np.uint32) + 0x8000, 16
            ).astype(np.uint16).view(ml_dtypes.bfloat16),
            "scal": np.ascontiguousarray(scal),
            "qdiag": qd.view(ml_dtypes.bfloat16),
        })
    return in_maps


def _bf16_to_f32(a):
    a = np.asarray(a)
    if a.dtype == np.float32:
        return a
    u = a.view(np.uint16).astype(np.uint32) << 16
    return u.view(np.float32)


def run(inputs, trace=False, **kwargs):
    """Run on 8 cores; returns (M [T, S_TOTAL] float32, BassKernelResults)."""
    nc = _get_nc()
    in_maps = _prep_inputs(**inputs)
    res = run_bass_kernel_spmd(nc, in_maps, core_ids=list(range(NCORES)),
                               trace=trace, **kwargs)
    cols = []
    for c in range(NCORES):
        mc = _bf16_to_f32(res.results[c]["m_out"])[:S_CORE]   # [S_CORE, T]
        cols.append(mc.T)
    M = np.concatenate(cols, axis=1)
    return np.ascontiguousarray(M, dtype=np.float32), res


def kernel(**inputs):
    M, _ = run(inputs)
    return M


# revision 6
# speedup vs baseline: 10.5055x; 1.0055x over previous
"""Trainium2 Bass kernel for the Covid compartment forecast model.

Math (per posterior sample s, forecast day t in [0, T)):
    growth[t,s] = r_t[t]**(1/T_serial[s]) * delta[s]
    A[t,s]      = A[t-1,s] * growth[t,s],  A[-1] = warmup[-1]
    M[t,s]      = sum_j A_full[J-1-j+t, s] * rho[s] * pi[j, s]

The sequential scan is replaced by its closed form
    A[t,s] = exp(b[s] + (t+1)*ln(delta[s]) + invT[s] * L[t]),
    L[t] = cumsum(ln r_t)[t],  b[s] = ln(warmup[-1, s])
so each 128-sample tile is fully parallel: one DVE scalar_tensor_tensor
builds the exponent, one ACT Exp (per-partition scale/bias) emits A in
bf16. The 32-tap FIR runs in bf16 split across engines by measured cost:
  PE   : diag(q_j) matmuls accumulated in PSUM        (~432ns/tap)
  ACT  : scaled copies B_j = A_shift * q_j            (~1131ns/tap)
  DVE  : tensor_scalar product + tensor_tensor adds   (~928ns/tap)
ACT products are pair-summed on DVE, then everything funnels through
identity matmuls into the same PSUM accumulation. M leaves as bf16 in
[sample, day] layout; the host does the final transpose/concat.

Sharding: posterior-sample dimension S across 8 cores (data parallel).
"""

import numpy as np

import concourse.bacc as bacc
import concourse.bass as bass
import concourse.mybir as mybir
import concourse.tile as tile
from concourse.bass_utils import run_bass_kernel_spmd

F32 = mybir.dt.float32
BF16 = mybir.dt.bfloat16
I32 = mybir.dt.int32
AF = mybir.ActivationFunctionType
OP = mybir.AluOpType

T = 1024
J = 32
S_TOTAL = 50000
NCORES = 8
P = 128
S_CORE = S_TOTAL // NCORES           # 6250
NTILES = (S_CORE + P - 1) // P       # 49
S_PAD = NTILES * P                   # 6272

# Tap -> engine. Offsets are 31-j; DVE taps take odd j (even offsets,
# needed for the 4x tensor_scalar perf mode on bf16). PE diag weight
# matrices are prebuilt on the host and DMA'd (DMA engines are idle).
PE_TAPS = tuple(range(0, 19))
ACT_TAPS = (20, 22, 24, 26, 28, 30)
DVE_TAPS = (19, 21, 23, 25, 27, 29, 31)


def build():
    taps = sorted(PE_TAPS + ACT_TAPS + DVE_TAPS)
    assert taps == list(range(J))

    nc = bacc.Bacc("TRN2", target_bir_lowering=False, debug=False,
                   num_devices=NCORES)
    it1 = nc.dram_tensor("iota1", [1, T], F32, kind="ExternalInput").ap()
    lc = nc.dram_tensor("lcum", [1, T], F32, kind="ExternalInput").ap()
    q = nc.dram_tensor("q", [S_PAD, J], F32, kind="ExternalInput").ap()
    wu = nc.dram_tensor("wu", [S_PAD, J], BF16, kind="ExternalInput").ap()
    sc = nc.dram_tensor("scal", [S_PAD, 4], F32, kind="ExternalInput").ap()
    qd = nc.dram_tensor("qdiag", [S_PAD, len(PE_TAPS) * P], BF16,
                        kind="ExternalInput").ap()
    m = nc.dram_tensor("m_out", [S_PAD, T], BF16, kind="ExternalOutput").ap()

    def bcast(a):
        return bass.AP(tensor=a.tensor, offset=a.offset, ap=[[0, P], [1, T]])

    with tile.TileContext(nc) as tc:
        with (
            tc.tile_pool(name="singles", bufs=1) as singles,
            tc.tile_pool(name="loads", bufs=4) as loads,
            tc.tile_pool(name="argp", bufs=3) as argp,
            tc.tile_pool(name="apool", bufs=3) as apool,
            tc.tile_pool(name="bpool", bufs=4) as bpool,
            tc.tile_pool(name="cpool", bufs=3) as cpool,
            tc.tile_pool(name="mdp", bufs=2) as mdp,
            tc.tile_pool(name="msb", bufs=3) as msb,
            tc.tile_pool(name="mpsum", bufs=4, space="PSUM") as mpsum,
        ):
            # ---- one-time setup ----
            iota_t = singles.tile([P, P], I32)
            nc.gpsimd.iota(iota_t, pattern=[[1, P]], base=0,
                           channel_multiplier=-1)
            ident = singles.tile([P, P], BF16)
            nc.vector.tensor_scalar(out=ident, in0=iota_t, scalar1=0,
                                    scalar2=None, op0=OP.is_equal)
            it_bc = singles.tile([P, T], F32)
            nc.sync.dma_start(out=it_bc, in_=bcast(it1))
            L_bc = singles.tile([P, T], F32)
            nc.sync.dma_start(out=L_bc, in_=bcast(lc))

            # ---- per sample-tile ----
            for i in range(NTILES):
                s0 = i * P

                qt = loads.tile([P, J], F32, tag="q")
                nc.sync.dma_start(out=qt, in_=q[s0:s0 + P, :])
                sct = loads.tile([P, 4], F32, tag="sc")
                nc.sync.dma_start(out=sct, in_=sc[s0:s0 + P, :])
                dgs = loads.tile([P, len(PE_TAPS) * P], BF16, tag="dgs")
                nc.sync.dma_start(out=dgs, in_=qd[s0:s0 + P, :])

                # exponent: arg[s,t] = (t+1)*w[s] + L[t]   (w = lnd*T_serial)
                argt = argp.tile([P, T], F32)
                nc.vector.scalar_tensor_tensor(
                    out=argt, in0=it_bc, scalar=sct[:, 0:1], in1=L_bc,
                    op0=OP.mult, op1=OP.add)

                # A_full[:, 0:J] = warmup (DMA, bf16), A_full[:, J:] = exp(...)
                A = apool.tile([P, J + T], BF16)
                nc.sync.dma_start(out=A[:, 0:J], in_=wu[s0:s0 + P, :])
                nc.scalar.activation(out=A[:, J:J + T], in_=argt, func=AF.Exp,
                                     bias=sct[:, 2:3], scale=sct[:, 1:2])

                # ---- FIR: M[t] = sum_j q[j] * A_full[31-j+t] ----
                Mp = mpsum.tile([P, T], F32)
                started = [False, False]

                def pe_acc(lhsT, rhs_base, stop=False):
                    for c in range(2):
                        lo = c * 512
                        nc.tensor.matmul(
                            Mp[:, lo:lo + 512], lhsT,
                            rhs_base[:, lo:lo + 512],
                            start=not started[c], stop=stop)
                        started[c] = True

                for k, j in enumerate(PE_TAPS):
                    pe_acc(dgs[:, k * P:(k + 1) * P],
                           A[:, J - 1 - j:J - 1 - j + T])

                # ACT taps -> pairwise sums on DVE -> PE merge
                Bs = []
                for j in ACT_TAPS:
                    B = bpool.tile([P, T], BF16, tag="b")
                    nc.scalar.activation(out=B,
                                         in_=A[:, J - 1 - j:J - 1 - j + T],
                                         func=AF.Copy, scale=qt[:, j:j + 1])
                    Bs.append(B)
                lastC = None
                for k in range(0, len(Bs), 2):
                    C = cpool.tile([P, T], BF16, tag=f"c{k}")
                    nc.vector.tensor_tensor(out=C, in0=Bs[k], in1=Bs[k + 1],
                                            op=OP.add)
                    if k + 2 >= len(Bs):
                        lastC = C
                    else:
                        pe_acc(ident, C)

                # DVE taps: TS products + balanced tree of TT adds
                prods = []
                for k, j in enumerate(DVE_TAPS):
                    Bt = mdp.tile([P, T], BF16, tag=f"bt{k}")
                    nc.vector.tensor_scalar(out=Bt,
                                            in0=A[:, J - 1 - j:J - 1 - j + T],
                                            scalar1=qt[:, j:j + 1],
                                            scalar2=None, op0=OP.mult)
                    prods.append(Bt)
                lvl = 0
                while len(prods) > 1:
                    nxt = []
                    for k in range(0, len(prods) - 1, 2):
                        Sm = mdp.tile([P, T], BF16, tag=f"s{lvl}{k}")
                        nc.vector.tensor_tensor(out=Sm, in0=prods[k],
                                                in1=prods[k + 1], op=OP.add)
                        nxt.append(Sm)
                    if len(prods) % 2:
                        nxt.append(prods[-1])
                    prods = nxt
                    lvl += 1
                Md = prods[0]
                # fold the last ACT pair into Md on DVE: one less PE merge
                Mf = mdp.tile([P, T], BF16, tag="mf")
                nc.vector.tensor_tensor(out=Mf, in0=Md, in1=lastC, op=OP.add)
                pe_acc(ident, Mf, stop=True)

                # PSUM -> SBUF bf16: one full-width ACT copy
                M_sb = msb.tile([P, T], BF16)
                nc.scalar.activation(out=M_sb, in_=Mp, func=AF.Copy)
                nc.sync.dma_start(out=m[s0:s0 + P, :], in_=M_sb)

    nc.compile()
    return nc


_NC_CACHE = {}


def _get_nc():
    key = (S_PAD, PE_TAPS, ACT_TAPS, DVE_TAPS)
    if key not in _NC_CACHE:
        _NC_CACHE[key] = build()
    return _NC_CACHE[key]


def _prep_inputs(r_t, warmup_A, delta, T_serial, rho_M, pi_M):
    """Host-side parameter prep + per-core sharding along S."""
    r_t = np.asarray(r_t, dtype=np.float32)
    warmup_A = np.asarray(warmup_A, dtype=np.float32)
    delta = np.asarray(delta, dtype=np.float32)
    T_serial = np.asarray(T_serial, dtype=np.float32)
    rho_M = np.asarray(rho_M, dtype=np.float32)
    pi_M = np.asarray(pi_M, dtype=np.float32)

    iota1 = np.arange(1, T + 1, dtype=np.float32).reshape(1, T)
    lcum = np.cumsum(np.log(r_t), dtype=np.float32).reshape(1, T)
    lnd = np.log(delta)
    q_full = (rho_M[None, :] * pi_M).T.astype(np.float32)       # [S, J]
    wu_full = warmup_A.T.astype(np.float32)                      # [S, J]
    w_full = lnd * T_serial
    invT_full = (1.0 / T_serial).astype(np.float32)
    b_full = np.log(warmup_A[-1]).astype(np.float32)

    q16 = np.right_shift(q_full.view(np.uint32) + 0x8000, 16).astype(np.uint16)

    pad = S_PAD - S_CORE
    in_maps = []
    for c in range(NCORES):
        lo, hi = c * S_CORE, (c + 1) * S_CORE

        def pad2(a, fill):
            return np.pad(a[lo:hi], ((0, pad), (0, 0)), constant_values=fill)

        scal = np.stack([w_full[lo:hi], invT_full[lo:hi], b_full[lo:hi],
                         np.zeros(S_CORE, np.float32)], axis=1)
        # padded lanes: w=-1, invT=1, b=0 -> A decays, q=0 -> M=0
        scal = np.pad(scal, ((0, pad), (0, 0)), constant_values=0.0)
        scal[S_CORE:, 0] = -1.0
        scal[S_CORE:, 1] = 1.0

        import ml_dtypes
        qd = np.zeros((S_PAD, len(PE_TAPS) * P), dtype=np.uint16)
        idx = np.arange(S_CORE)
        for k, j in enumerate(PE_TAPS):
            qd[idx, k * P + (idx % P)] = q16[lo:hi, j]
        in_maps.append({
            "iota1": iota1,
            "lcum": lcum,
            "q": pad2(q_full, 0.0),
            "wu": np.right_shift(
                pad2(wu_full, 1.0).view(np.uint32) + 0x8000, 16
            ).astype(np.uint16).view(ml_dtypes.bfloat16),
            "scal": np.ascontiguousarray(scal),
            "qdiag": qd.view(ml_dtypes.bfloat16),
        })
    return in_maps


def _bf16_to_f32(a):
    a = np.asarray(a)
    if a.dtype == np.float32:
        return a
    u = a.view(np.uint16).astype(np.uint32) << 16
    return u.view(np.float32)


def run(inputs, trace=False, **kwargs):
    """Run on 8 cores; returns (M [T, S_TOTAL] float32, BassKernelResults)."""
    nc = _get_nc()
    in_maps = _prep_inputs(**inputs)
    res = run_bass_kernel_spmd(nc, in_maps, core_ids=list(range(NCORES)),
                               trace=trace, **kwargs)
    cols = []
    for c in range(NCORES):
        mc = _bf16_to_f32(res.results[c]["m_out"])[:S_CORE]   # [S_CORE, T]
        cols.append(mc.T)
    M = np.concatenate(cols, axis=1)
    return np.ascontiguousarray(M, dtype=np.float32), res


def kernel(**inputs):
    M, _ = run(inputs)
    return M


# revision 7
# speedup vs baseline: 10.7735x; 1.0255x over previous
"""Trainium2 Bass kernel for the Covid compartment forecast model.

Math (per posterior sample s, forecast day t in [0, T)):
    growth[t,s] = r_t[t]**(1/T_serial[s]) * delta[s]
    A[t,s]      = A[t-1,s] * growth[t,s],  A[-1] = warmup[-1]
    M[t,s]      = sum_j A_full[J-1-j+t, s] * rho[s] * pi[j, s]

The sequential scan is replaced by its closed form
    A[t,s] = exp(b[s] + (t+1)*ln(delta[s]) + invT[s] * L[t]),
    L[t] = cumsum(ln r_t)[t],  b[s] = ln(warmup[-1, s])
so each 128-sample tile is fully parallel: one DVE scalar_tensor_tensor
builds the exponent, one ACT Exp (per-partition scale/bias) emits A in
bf16. The 32-tap FIR runs in bf16 split across engines by measured cost:
  PE   : diag(q_j) matmuls accumulated in PSUM        (~432ns/tap)
  ACT  : scaled copies B_j = A_shift * q_j            (~1131ns/tap)
  DVE  : tensor_scalar product + tensor_tensor adds   (~928ns/tap)
ACT products are pair-summed on DVE, then everything funnels through
identity matmuls into the same PSUM accumulation. M leaves as bf16 in
[sample, day] layout; the host does the final transpose/concat.

Sharding: posterior-sample dimension S across 8 cores (data parallel).
"""

import numpy as np

import concourse.bacc as bacc
import concourse.bass as bass
import concourse.mybir as mybir
import concourse.tile as tile
from concourse.bass_utils import run_bass_kernel_spmd

F32 = mybir.dt.float32
BF16 = mybir.dt.bfloat16
I32 = mybir.dt.int32
AF = mybir.ActivationFunctionType
OP = mybir.AluOpType

T = 1024
J = 32
S_TOTAL = 50000
NCORES = 8
P = 128
S_CORE = S_TOTAL // NCORES           # 6250
NTILES = (S_CORE + P - 1) // P       # 49
S_PAD = NTILES * P                   # 6272

# Tap -> engine. Offsets are 31-j; DVE taps take odd j (even offsets,
# needed for the 4x tensor_scalar perf mode on bf16). PE diag weight
# matrices are prebuilt on the host and DMA'd (DMA engines are idle).
PE_TAPS = tuple(range(0, 20))
ACT_TAPS = (20, 22, 24, 26, 28, 30)
DVE_TAPS = (21, 23, 25, 27, 29, 31)


def build():
    taps = sorted(PE_TAPS + ACT_TAPS + DVE_TAPS)
    assert taps == list(range(J))

    nc = bacc.Bacc("TRN2", target_bir_lowering=False, debug=False,
                   num_devices=NCORES)
    it1 = nc.dram_tensor("iota1", [1, T], F32, kind="ExternalInput").ap()
    lc = nc.dram_tensor("lcum", [1, T], F32, kind="ExternalInput").ap()
    q = nc.dram_tensor("q", [S_PAD, J], F32, kind="ExternalInput").ap()
    wu = nc.dram_tensor("wu", [S_PAD, J], BF16, kind="ExternalInput").ap()
    sc = nc.dram_tensor("scal", [S_PAD, 4], F32, kind="ExternalInput").ap()
    qd = nc.dram_tensor("qdiag", [S_PAD, len(PE_TAPS) * P], BF16,
                        kind="ExternalInput").ap()
    m = nc.dram_tensor("m_out", [S_PAD, T], BF16, kind="ExternalOutput").ap()

    def bcast(a):
        return bass.AP(tensor=a.tensor, offset=a.offset, ap=[[0, P], [1, T]])

    with tile.TileContext(nc) as tc:
        with (
            tc.tile_pool(name="singles", bufs=1) as singles,
            tc.tile_pool(name="loads", bufs=4) as loads,
            tc.tile_pool(name="argp", bufs=3) as argp,
            tc.tile_pool(name="apool", bufs=3) as apool,
            tc.tile_pool(name="bpool", bufs=4) as bpool,
            tc.tile_pool(name="cpool", bufs=3) as cpool,
            tc.tile_pool(name="mdp", bufs=2) as mdp,
            tc.tile_pool(name="msb", bufs=3) as msb,
            tc.tile_pool(name="mpsum", bufs=4, space="PSUM") as mpsum,
        ):
            # ---- one-time setup ----
            iota_t = singles.tile([P, P], I32)
            nc.gpsimd.iota(iota_t, pattern=[[1, P]], base=0,
                           channel_multiplier=-1)
            ident = singles.tile([P, P], BF16)
            nc.vector.tensor_scalar(out=ident, in0=iota_t, scalar1=0,
                                    scalar2=None, op0=OP.is_equal)
            it_bc = singles.tile([P, T], F32)
            nc.sync.dma_start(out=it_bc, in_=bcast(it1))
            L_bc = singles.tile([P, T], F32)
            nc.sync.dma_start(out=L_bc, in_=bcast(lc))

            # ---- per sample-tile ----
            for i in range(NTILES):
                s0 = i * P

                qt = loads.tile([P, J], F32, tag="q")
                nc.sync.dma_start(out=qt, in_=q[s0:s0 + P, :])
                sct = loads.tile([P, 4], F32, tag="sc")
                nc.sync.dma_start(out=sct, in_=sc[s0:s0 + P, :])
                dgs = loads.tile([P, len(PE_TAPS) * P], BF16, tag="dgs")
                nc.sync.dma_start(out=dgs, in_=qd[s0:s0 + P, :])

                # exponent: arg[s,t] = (t+1)*w[s] + L[t]   (w = lnd*T_serial)
                argt = argp.tile([P, T], F32)
                nc.vector.scalar_tensor_tensor(
                    out=argt, in0=it_bc, scalar=sct[:, 0:1], in1=L_bc,
                    op0=OP.mult, op1=OP.add)

                # A_full[:, 0:J] = warmup (DMA, bf16), A_full[:, J:] = exp(...)
                A = apool.tile([P, J + T], BF16)
                nc.sync.dma_start(out=A[:, 0:J], in_=wu[s0:s0 + P, :])
                nc.scalar.activation(out=A[:, J:J + T], in_=argt, func=AF.Exp,
                                     bias=sct[:, 2:3], scale=sct[:, 1:2])

                # ---- FIR: M[t] = sum_j q[j] * A_full[31-j+t] ----
                Mp = mpsum.tile([P, T], F32)
                started = [False, False]

                def pe_acc(lhsT, rhs_base, stop=False):
                    for c in range(2):
                        lo = c * 512
                        nc.tensor.matmul(
                            Mp[:, lo:lo + 512], lhsT,
                            rhs_base[:, lo:lo + 512],
                            start=not started[c], stop=stop)
                        started[c] = True

                for k, j in enumerate(PE_TAPS):
                    pe_acc(dgs[:, k * P:(k + 1) * P],
                           A[:, J - 1 - j:J - 1 - j + T])

                # ACT taps -> pairwise sums on DVE -> PE merge
                Bs = []
                for j in ACT_TAPS:
                    B = bpool.tile([P, T], BF16, tag="b")
                    nc.scalar.activation(out=B,
                                         in_=A[:, J - 1 - j:J - 1 - j + T],
                                         func=AF.Copy, scale=qt[:, j:j + 1])
                    Bs.append(B)
                Cs = []
                for k in range(0, len(Bs), 2):
                    C = cpool.tile([P, T], BF16, tag=f"c{k}")
                    nc.vector.tensor_tensor(out=C, in0=Bs[k], in1=Bs[k + 1],
                                            op=OP.add)
                    Cs.append(C)
                CC = cpool.tile([P, T], BF16, tag="cc")
                nc.vector.tensor_tensor(out=CC, in0=Cs[0], in1=Cs[1],
                                        op=OP.add)
                pe_acc(ident, CC)
                lastC = Cs[2]

                # DVE taps: TS products + balanced tree of TT adds
                prods = []
                for k, j in enumerate(DVE_TAPS):
                    Bt = mdp.tile([P, T], BF16, tag=f"bt{k}")
                    nc.vector.tensor_scalar(out=Bt,
                                            in0=A[:, J - 1 - j:J - 1 - j + T],
                                            scalar1=qt[:, j:j + 1],
                                            scalar2=None, op0=OP.mult)
                    prods.append(Bt)
                lvl = 0
                while len(prods) > 1:
                    nxt = []
                    for k in range(0, len(prods) - 1, 2):
                        Sm = mdp.tile([P, T], BF16, tag=f"s{lvl}{k}")
                        nc.vector.tensor_tensor(out=Sm, in0=prods[k],
                                                in1=prods[k + 1], op=OP.add)
                        nxt.append(Sm)
                    if len(prods) % 2:
                        nxt.append(prods[-1])
                    prods = nxt
                    lvl += 1
                Md = prods[0]
                # fold the last ACT pair into Md on DVE: one less PE merge
                Mf = mdp.tile([P, T], BF16, tag="mf")
                nc.vector.tensor_tensor(out=Mf, in0=Md, in1=lastC, op=OP.add)
                pe_acc(ident, Mf, stop=True)

                # PSUM -> SBUF bf16: one full-width ACT copy
                M_sb = msb.tile([P, T], BF16)
                nc.scalar.activation(out=M_sb, in_=Mp, func=AF.Copy)
                nc.sync.dma_start(out=m[s0:s0 + P, :], in_=M_sb)

    nc.compile()
    return nc


_NC_CACHE = {}


def _get_nc():
    key = (S_PAD, PE_TAPS, ACT_TAPS, DVE_TAPS)
    if key not in _NC_CACHE:
        _NC_CACHE[key] = build()
    return _NC_CACHE[key]


def _prep_inputs(r_t, warmup_A, delta, T_serial, rho_M, pi_M):
    """Host-side parameter prep + per-core sharding along S."""
    r_t = np.asarray(r_t, dtype=np.float32)
    warmup_A = np.asarray(warmup_A, dtype=np.float32)
    delta = np.asarray(delta, dtype=np.float32)
    T_serial = np.asarray(T_serial, dtype=np.float32)
    rho_M = np.asarray(rho_M, dtype=np.float32)
    pi_M = np.asarray(pi_M, dtype=np.float32)

    iota1 = np.arange(1, T + 1, dtype=np.float32).reshape(1, T)
    lcum = np.cumsum(np.log(r_t), dtype=np.float32).reshape(1, T)
    lnd = np.log(delta)
    q_full = (rho_M[None, :] * pi_M).T.astype(np.float32)       # [S, J]
    wu_full = warmup_A.T.astype(np.float32)                      # [S, J]
    w_full = lnd * T_serial
    invT_full = (1.0 / T_serial).astype(np.float32)
    b_full = np.log(warmup_A[-1]).astype(np.float32)

    q16 = np.right_shift(q_full.view(np.uint32) + 0x8000, 16).astype(np.uint16)

    pad = S_PAD - S_CORE
    in_maps = []
    for c in range(NCORES):
        lo, hi = c * S_CORE, (c + 1) * S_CORE

        def pad2(a, fill):
            return np.pad(a[lo:hi], ((0, pad), (0, 0)), constant_values=fill)

        scal = np.stack([w_full[lo:hi], invT_full[lo:hi], b_full[lo:hi],
                         np.zeros(S_CORE, np.float32)], axis=1)
        # padded lanes: w=-1, invT=1, b=0 -> A decays, q=0 -> M=0
        scal = np.pad(scal, ((0, pad), (0, 0)), constant_values=0.0)
        scal[S_CORE:, 0] = -1.0
        scal[S_CORE:, 1] = 1.0

        import ml_dtypes
        qd = np.zeros((S_PAD, len(PE_TAPS) * P), dtype=np.uint16)
        idx = np.arange(S_CORE)
        for k, j in enumerate(PE_TAPS):
            qd[idx, k * P + (idx % P)] = q16[lo:hi, j]
        in_maps.append({
            "iota1": iota1,
            "lcum": lcum,
            "q": pad2(q_full, 0.0),
            "wu": np.right_shift(
                pad2(wu_full, 1.0).view(np.uint32) + 0x8000, 16
            ).astype(np.uint16).view(ml_dtypes.bfloat16),
            "scal": np.ascontiguousarray(scal),
            "qdiag": qd.view(ml_dtypes.bfloat16),
        })
    return in_maps


def _bf16_to_f32(a):
    a = np.asarray(a)
    if a.dtype == np.float32:
        return a
    u = a.view(np.uint16).astype(np.uint32) << 16
    return u.view(np.float32)


def run(inputs, trace=False, **kwargs):
    """Run on 8 cores; returns (M [T, S_TOTAL] float32, BassKernelResults)."""
    nc = _get_nc()
    in_maps = _prep_inputs(**inputs)
    res = run_bass_kernel_spmd(nc, in_maps, core_ids=list(range(NCORES)),
                               trace=trace, **kwargs)
    cols = []
    for c in range(NCORES):
        mc = _bf16_to_f32(res.results[c]["m_out"])[:S_CORE]   # [S_CORE, T]
        cols.append(mc.T)
    M = np.concatenate(cols, axis=1)
    return np.ascontiguousarray(M, dtype=np.float32), res


def kernel(**inputs):
    M, _ = run(inputs)
    return M
